# revision 31
# baseline (speedup 1.0000x reference)
"""TRN2 Bass kernel for nn_FP8LinearWrapper: y = x @ (w_fp8 * inv_scale).T + bias.

Strategy (8 NeuronCores, SPMD):
  - Data-parallel over the flattened token dim: x [4,2048,4096] -> [8192,4096],
    1024 rows per core. Weights/bias replicated to every core.
  - Per core: SINGLE-pass bf16 matmul (x cast to bf16 on device) against the
    exactly-dequantized bf16 weight, accumulating in fp32 PSUM. Error budget:
    x's bf16 rounding alone -> rel_absmax ~1.7e-3 (measured vs fp32 reference),
    well under the 2e-2 gate; the fp8 weight dequant is exact in bf16.
  - The fp8 weight bytes are jax float8_e4m3fn (max 448). TRN2's fp8e4 decode
    is IEEE e4m3 (max 240), so the host re-encodes each byte via a LUT to the
    e4m3 bits of (value/2) - exact for all normals - and the kernel folds the
    missing *2 into the output scale. Weights stay 1 byte; all arithmetic
    (dequant cast, transpose of x, matmul, scale, bias) runs on device.
  - x is transposed on device via PE-transpose (contraction dim must be on
    SBUF partitions for both matmul operands). w is passed pre-transposed /
    pre-blocked (weight layout prep, as for any serving stack).

Timing-critical structure (PE roofline: 2048 matmuls x 216 ns N=512 bf16
streaming cadence + 256 PE-transposes x ~108 ns ~ 470 us/core):
  - Phase A: a software-pipelined chunk stream: per 1024-col chunk, 8 PE
    transposes land in ONE 2-bank PSUM tile, one merged scalar copy (f32 psum
    -> bf16 xt, contiguous both sides) retires them, and the o-block-0 matmuls
    for chunk g-2 run in between (2-chunk skew) so the PE never waits on the
    scalar engine. o-block-0 weight dequants are interleaved into the first
    chunks' scalar stream to keep mt0 dense (HAM clock gate stays 8/8).
  - O-blocks 1..7: m-tile PAIRS share one 2-bank PSUM tile (2 x 32
    accumulating matmuls) with a single fused (psum * 2*inv_scale) + bias DVE
    eviction covering both banks and ONE pair-merged y DMA (256 rows).
"""

import os
import sys

for _p in (
    "/opt/trn_rl_repo",
    "/root/.axon_site",
    "/root/.axon_site/_ro/trn_rl_repo",
    "/root/.axon_site/_ro/pypackages",
):
    if os.path.isdir(_p) and _p not in sys.path:
        sys.path.append(_p)

import numpy as np
import ml_dtypes

B, S, DI, DO = 4, 2048, 4096, 4096
NCORES = 8
M = B * S            # 8192
MC = M // NCORES     # 1024 rows per core
P = 128
KT = DI // P         # 32 k-tiles
MT = MC // P         # 8 m-tiles per core
OBW = 512            # o-block width
OB = DO // OBW       # 8 o-blocks
WCK = 4              # k-tiles per weight chunk
WCH = KT // WCK      # 8 weight chunks per o-block
XC = 4               # 1024-col x chunks per m-tile
NCH = MT * XC        # 32 chunks total
SKEW = 2             # ob0 matmuls lag the transpose stream by this many chunks

_STATE = {}


def _build_program():
    import concourse.bass as bass
    import concourse.mybir as mybir
    import concourse.tile as tile
    from concourse import bacc
    from concourse.masks import make_identity

    dt = mybir.dt
    F32, BF16, FP8 = dt.float32, dt.bfloat16, dt.float8e4

    nc = bacc.Bacc(target_bir_lowering=False)

    x_in = nc.dram_tensor("x", [MC, DI], F32, kind="ExternalInput")
    w_in = nc.dram_tensor("w", [OB, P, KT, OBW], FP8, kind="ExternalInput")
    s_in = nc.dram_tensor("s", [P, 1], F32, kind="ExternalInput")
    b_in = nc.dram_tensor("b", [P, DO], F32, kind="ExternalInput")
    y_out = nc.dram_tensor("y", [MC, DO], F32, kind="ExternalOutput")

    with tile.TileContext(nc) as tc:
        with (
            tc.tile_pool(name="const", bufs=1) as const,
            tc.tile_pool(name="xt_pool", bufs=1) as xt_pool,
            tc.tile_pool(name="xin_pool", bufs=8) as xin_pool,
            tc.tile_pool(name="xbf_pool", bufs=4) as xbf_pool,
            tc.tile_pool(name="w8_pool", bufs=10) as w8_pool,
            tc.tile_pool(name="wb_pool", bufs=10) as wb_pool,
            tc.tile_pool(name="bias_pool", bufs=2) as bias_pool,
            tc.tile_pool(name="out_pool", bufs=4) as out_pool,
            tc.tile_pool(name="tp_ps_pool", bufs=2, space="PSUM") as tp_ps_pool,
            tc.tile_pool(name="warm_ps_pool", bufs=1, space="PSUM") as warm_ps_pool,
            tc.tile_pool(name="mm_ps_pool", bufs=2, space="PSUM") as mm_ps_pool,
        ):
            # x DMAs for the first chunks go first on the sync queue: they
            # head the critical chain (DMA -> cast -> transpose -> copy -> mm)
            first_xins = {}
            for g in range(2):
                xin = xin_pool.tile([P, 1024], F32, name=f"xin_{g}", tag="xin")
                nc.sync.dma_start(out=xin, in_=x_in[0:P, g * 1024:(g + 1) * 1024])
                first_xins[g] = xin

            ident = const.tile([P, P], F32)
            make_identity(nc, ident)
            ident_bf = const.tile([P, P], BF16)
            nc.vector.tensor_copy(ident_bf, ident)
            s_t = const.tile([P, 1], F32)
            nc.sync.dma_start(out=s_t, in_=s_in[:, :])
            s2 = const.tile([P, 1], F32)
            nc.scalar.mul(s2, s_t, 2.0)  # fold back the /2 from the fp8 re-encode
            garbage = const.tile([P, P], BF16)  # HAM warm-up fodder only
            nc.gpsimd.memset(garbage, 0)

            # resident transposed x: [128 d, mt, kt, 128 m] bf16
            xt = xt_pool.tile([P, MT, KT, P], BF16)

            def load_wchunk(ob, c):
                w8c = w8_pool.tile([P, WCK, OBW], FP8, name=f"w8_{ob}_{c}", tag="w8")
                nc.sync.dma_start(out=w8c, in_=w_in[ob, :, c * WCK:(c + 1) * WCK, :])
                wbc = wb_pool.tile([P, WCK, OBW], BF16, name=f"wb_{ob}_{c}", tag="wb")
                nc.scalar.copy(wbc, w8c)
                return wbc

            def emit_mms(ps_slice, mt, wchunks, kts):
                for kt in kts:
                    wb_sl = wchunks[kt // WCK][:, kt % WCK, :]
                    nc.tensor.matmul(
                        ps_slice, xt[:, mt, kt, :], wb_sl,
                        start=(kt == 0), stop=(kt == KT - 1),
                        skip_group_check=True,
                    )

            def evict(ps, bias_t, y_ap):
                # fused (psum * s2) + bias -> SBUF, then one DMA out; for
                # pairs, y_ap covers 256 rows and both banks go in one DMA
                w = ps.shape[-1]
                out_sb = out_pool.tile([P, w], F32, tag="out")
                nc.vector.scalar_tensor_tensor(
                    out_sb, ps, s2[:, :], bias_t,
                    mybir.AluOpType.mult, mybir.AluOpType.add,
                )
                if w == OBW:
                    nc.sync.dma_start(out=y_ap, in_=out_sb)
                else:
                    nc.sync.dma_start(
                        out=y_ap.rearrange("(h r) c -> r h c", h=2),
                        in_=out_sb.rearrange("p (h c) -> p h c", h=2),
                    )

            # ---- Phase A: pipelined chunk stream; ob0 matmuls lag the
            # transpose+copy stream by SKEW chunks ----
            bias0 = bias_pool.tile([P, OBW], F32, name="bias_0", tag="bias")
            nc.sync.dma_start(out=bias0, in_=b_in[:, 0:OBW])
            wch0 = []
            ps0 = {}

            # HAM warm-up: dummy transposes of the identity keep the PE busy
            # while the first x chunk is still in flight, so the clock gate
            # ramps to 8/8 before the real stream starts instead of ~25us in
            # HAM warm-up: dependency-free dummy transposes (garbage data,
            # output discarded) start right after the engine preamble and
            # keep the PE busy so the clock gate ramps to 8/8 early
            tp_w = warm_ps_pool.tile([P, 1024], BF16, name="tp_warm", tag="warm")

            def warm(n):
                # dependency-free filler transposes (WAW-only on tp_w): they
                # never wait, so they pad PE bubbles without blocking real work
                for i in range(n):
                    nc.tensor.matmul(
                        tp_w[:, (i % 8) * P:(i % 8 + 1) * P], garbage, garbage,
                        is_transpose=True, skip_group_check=True,
                    )
            warm(48)
            # first two ob0 weight chunks ahead of the loop: their dequants
            # precede the first transpose copy in the scalar queue
            wch0.append(load_wchunk(0, 0))
            wch0.append(load_wchunk(0, 1))

            def prep_chunk(g):
                mt, c = divmod(g, XC)
                if g in first_xins:
                    xin = first_xins[g]
                else:
                    xin = xin_pool.tile([P, 1024], F32, name=f"xin_{g}", tag="xin")
                    nc.sync.dma_start(
                        out=xin, in_=x_in[mt * P:(mt + 1) * P, c * 1024:(c + 1) * 1024]
                    )
                # DVE pre-cast to bf16: the PE transpose then streams 16-bit
                # (2 cols/cycle) and its PSUM tile is 1 bank instead of 2
                xbf = xbf_pool.tile([P, 1024], BF16, name=f"xbf_{g}", tag="xbf")
                nc.vector.tensor_copy(xbf, xin)
                tp = tp_ps_pool.tile([P, 1024], BF16, name=f"tp_{g}", tag="tp")
                for kk in range(8):
                    nc.tensor.matmul(
                        tp[:, kk * P:(kk + 1) * P], xbf[:, kk * P:(kk + 1) * P],
                        ident_bf, is_transpose=True, skip_group_check=True,
                    )
                # one merged copy: [128, 1024] f32 PSUM -> bf16 xt (contiguous)
                nc.scalar.copy(xt[:, mt, c * 8:(c + 1) * 8, :], tp)
                # interleave ob0 weight dequants into the early scalar stream
                if g < WCH // 2 - 1:
                    wch0.append(load_wchunk(0, 2 * g + 2))
                    wch0.append(load_wchunk(0, 2 * g + 3))

            def mm_chunk(g):
                mt, c = divmod(g, XC)
                if c == 0:
                    ps0[mt] = mm_ps_pool.tile([P, OBW], F32, name=f"ps_0_{mt}", tag="ps")
                emit_mms(ps0[mt], mt, wch0, range(c * 8, (c + 1) * 8))
                if c == XC - 1:
                    evict(ps0.pop(mt), bias0, y_out[mt * P:(mt + 1) * P, 0:OBW])

            for g in range(NCH + SKEW):
                if g < NCH:
                    prep_chunk(g)
                    if g < 8:
                        # pad the pipeline-ramp bubbles so the clock gate
                        # never dips while the first chunks stream in
                        warm(8)
                if g >= SKEW:
                    mm_chunk(g - SKEW)

            # ---- Phase B: o-blocks 1..7, m-tile pairs share one 2-bank PSUM
            # tile and one fused eviction + one pair-merged y DMA ----
            for ob in range(1, OB):
                wchunks = [load_wchunk(ob, c) for c in range(WCH)]
                bias2 = bias_pool.tile([P, 2 * OBW], F32, name=f"bias2_{ob}", tag="bias2")
                for h in range(2):
                    nc.sync.dma_start(
                        out=bias2[:, h * OBW:(h + 1) * OBW],
                        in_=b_in[:, ob * OBW:(ob + 1) * OBW],
                    )
                for mt0 in range(0, MT, 2):
                    if ob == OB - 1 and mt0 == MT - 2:
                        # last pair: two single-bank groups so the final
                        # eviction + y DMA chain at the drain is half as long
                        for mt in (mt0, mt0 + 1):
                            ps = mm_ps_pool.tile([P, OBW], F32, name=f"ps_{ob}_{mt}", tag="ps")
                            emit_mms(ps, mt, wchunks, range(KT))
                            evict(ps, bias2[:, 0:OBW],
                                  y_out[mt * P:(mt + 1) * P, ob * OBW:(ob + 1) * OBW])
                    else:
                        ps = mm_ps_pool.tile([P, 2 * OBW], F32, name=f"ps_{ob}_{mt0}", tag="ps")
                        emit_mms(ps[:, 0:OBW], mt0, wchunks, range(KT))
                        emit_mms(ps[:, OBW:2 * OBW], mt0 + 1, wchunks, range(KT))
                        evict(ps, bias2,
                              y_out[mt0 * P:(mt0 + 2) * P, ob * OBW:(ob + 1) * OBW])

    nc.finalize()
    return nc


def _get_program():
    if "nc" not in _STATE:
        _STATE["nc"] = _build_program()
    return _STATE["nc"]


def _prep_weights(weight_fp8):
    """Re-encode jax e4m3fn bytes as IEEE-e4m3 bytes of value/2 (exact for
    normals), transpose to [d, o], and block to [ob, p, kt, obw] so each
    o-block DMA reads 2KB-contiguous per-partition lines."""
    bits = np.arange(256, dtype=np.uint8)
    vals = bits.view(ml_dtypes.float8_e4m3fn).astype(np.float32) * 0.5
    lut = vals.astype(ml_dtypes.float8_e4m3).view(np.uint8)

    wb = np.asarray(weight_fp8).view(np.uint8)          # [DO, DI]
    w2t = np.ascontiguousarray(lut[wb].T)               # [DI, DO]
    w_pre = np.ascontiguousarray(
        w2t.reshape(KT, P, OB, OBW).transpose(2, 1, 0, 3)
    )                                                   # [OB, P, KT, OBW]
    return w_pre.view(ml_dtypes.float8_e4m3)


def kernel(x, weight_fp8, weight_inv_scale, bias):
    from concourse.bass_utils import run_bass_kernel_spmd

    try:
        import jax
        jax.config.update("jax_compilation_cache_dir", "/tmp/jax_neff_cache")
        jax.config.update("jax_persistent_cache_min_entry_size_bytes", 0)
        jax.config.update("jax_persistent_cache_min_compile_time_secs", 0.0)
    except Exception:
        pass

    nc = _get_program()

    x_np = np.asarray(x, dtype=np.float32).reshape(M, DI)
    w_pre = _prep_weights(weight_fp8)
    s_b = np.ascontiguousarray(
        np.broadcast_to(
            np.asarray(weight_inv_scale, dtype=np.float32).reshape(1, 1), (P, 1)
        )
    )
    b_b = np.ascontiguousarray(
        np.broadcast_to(np.asarray(bias, dtype=np.float32), (P, DO))
    )

    core_ids = list(range(NCORES))
    in_maps = [
        {"x": x_np[c * MC:(c + 1) * MC], "w": w_pre, "s": s_b, "b": b_b}
        for c in core_ids
    ]

    last_err = None
    for _attempt in range(3):
        try:
            res = run_bass_kernel_spmd(nc, in_maps, core_ids)
            break
        except Exception as e:  # device wedge (NRT_EXEC_UNIT_UNRECOVERABLE): reset + retry
            last_err = e
            try:
                import jax
                import time
                jax.clear_backends()
                time.sleep(3.0)
            except Exception:
                pass
    else:
        raise last_err

    y = np.concatenate([res.results[c]["y"] for c in core_ids], axis=0)
    return y.reshape(B, S, DO)


# revision 33
# speedup vs baseline: 1.0063x; 1.0063x over previous
"""TRN2 Bass kernel for nn_FP8LinearWrapper: y = x @ (w_fp8 * inv_scale).T + bias.

Strategy (8 NeuronCores, SPMD):
  - Data-parallel over the flattened token dim: x [4,2048,4096] -> [8192,4096],
    1024 rows per core. Weights/bias replicated to every core.
  - Per core: SINGLE-pass bf16 matmul (x cast to bf16 on device) against the
    exactly-dequantized bf16 weight, accumulating in fp32 PSUM. Error budget:
    x's bf16 rounding alone -> rel_absmax ~1.7e-3 (measured vs fp32 reference),
    well under the 2e-2 gate; the fp8 weight dequant is exact in bf16.
  - The fp8 weight bytes are jax float8_e4m3fn (max 448). TRN2's fp8e4 decode
    is IEEE e4m3 (max 240), so the host re-encodes each byte via a LUT to the
    e4m3 bits of (value/2) - exact for all normals - and the kernel folds the
    missing *2 into the output scale. Weights stay 1 byte; all arithmetic
    (dequant cast, transpose of x, matmul, scale, bias) runs on device.
  - x is transposed on device via PE-transpose (contraction dim must be on
    SBUF partitions for both matmul operands). w is passed pre-transposed /
    pre-blocked (weight layout prep, as for any serving stack).

Timing-critical structure (PE roofline: 2048 matmuls x 216 ns N=512 bf16
streaming cadence + 256 PE-transposes x ~108 ns ~ 470 us/core):
  - Phase A: a software-pipelined chunk stream: per 1024-col chunk, 8 PE
    transposes land in ONE 2-bank PSUM tile, one merged scalar copy (f32 psum
    -> bf16 xt, contiguous both sides) retires them, and the o-block-0 matmuls
    for chunk g-2 run in between (2-chunk skew) so the PE never waits on the
    scalar engine. o-block-0 weight dequants are interleaved into the first
    chunks' scalar stream to keep mt0 dense (HAM clock gate stays 8/8).
  - O-blocks 1..7: m-tile PAIRS share one 2-bank PSUM tile (2 x 32
    accumulating matmuls) with a single fused (psum * 2*inv_scale) + bias DVE
    eviction covering both banks and ONE pair-merged y DMA (256 rows).
"""

import os
import sys

for _p in (
    "/opt/trn_rl_repo",
    "/root/.axon_site",
    "/root/.axon_site/_ro/trn_rl_repo",
    "/root/.axon_site/_ro/pypackages",
):
    if os.path.isdir(_p) and _p not in sys.path:
        sys.path.append(_p)

import numpy as np
import ml_dtypes

B, S, DI, DO = 4, 2048, 4096, 4096
NCORES = 8
M = B * S            # 8192
MC = M // NCORES     # 1024 rows per core
P = 128
KT = DI // P         # 32 k-tiles
MT = MC // P         # 8 m-tiles per core
OBW = 512            # o-block width
OB = DO // OBW       # 8 o-blocks
WCK = 4              # k-tiles per weight chunk
WCH = KT // WCK      # 8 weight chunks per o-block
XC = 4               # 1024-col x chunks per m-tile
NCH = MT * XC        # 32 chunks total
SKEW = 2             # ob0 matmuls lag the transpose stream by this many chunks

_STATE = {}


def _build_program():
    import concourse.bass as bass
    import concourse.mybir as mybir
    import concourse.tile as tile
    from concourse import bacc
    from concourse.masks import make_identity

    dt = mybir.dt
    F32, BF16, FP8 = dt.float32, dt.bfloat16, dt.float8e4

    nc = bacc.Bacc(target_bir_lowering=False)

    x_in = nc.dram_tensor("x", [MC, DI], F32, kind="ExternalInput")
    w_in = nc.dram_tensor("w", [OB, P, KT, OBW], FP8, kind="ExternalInput")
    s_in = nc.dram_tensor("s", [P, 1], F32, kind="ExternalInput")
    b_in = nc.dram_tensor("b", [P, DO], F32, kind="ExternalInput")
    y_out = nc.dram_tensor("y", [MC, DO], F32, kind="ExternalOutput")

    with tile.TileContext(nc) as tc:
        with (
            tc.tile_pool(name="const", bufs=1) as const,
            tc.tile_pool(name="xt_pool", bufs=1) as xt_pool,
            tc.tile_pool(name="xin_pool", bufs=8) as xin_pool,
            tc.tile_pool(name="xbf_pool", bufs=4) as xbf_pool,
            tc.tile_pool(name="w8_pool", bufs=10) as w8_pool,
            tc.tile_pool(name="wb_pool", bufs=10) as wb_pool,
            tc.tile_pool(name="bias_pool", bufs=2) as bias_pool,
            tc.tile_pool(name="out_pool", bufs=4) as out_pool,
            tc.tile_pool(name="tp_ps_pool", bufs=2, space="PSUM") as tp_ps_pool,
            tc.tile_pool(name="warm_ps_pool", bufs=1, space="PSUM") as warm_ps_pool,
            tc.tile_pool(name="mm_ps_pool", bufs=2, space="PSUM") as mm_ps_pool,
        ):
            # x DMAs for the first chunks go first on the sync queue: they
            # head the critical chain (DMA -> cast -> transpose -> copy -> mm)
            first_xins = {}
            for g in range(2):
                xin = xin_pool.tile([P, 1024], F32, name=f"xin_{g}", tag="xin")
                nc.sync.dma_start(out=xin, in_=x_in[0:P, g * 1024:(g + 1) * 1024])
                first_xins[g] = xin

            ident = const.tile([P, P], F32)
            make_identity(nc, ident)
            ident_bf = const.tile([P, P], BF16)
            nc.vector.tensor_copy(ident_bf, ident)
            s_t = const.tile([P, 1], F32)
            nc.sync.dma_start(out=s_t, in_=s_in[:, :])
            s2 = const.tile([P, 1], F32)
            nc.scalar.mul(s2, s_t, 2.0)  # fold back the /2 from the fp8 re-encode
            garbage = const.tile([P, P], BF16)  # HAM warm-up fodder only
            nc.gpsimd.memset(garbage, 0)

            # resident transposed x: [128 d, mt, kt, 128 m] bf16
            xt = xt_pool.tile([P, MT, KT, P], BF16)

            def load_wchunk(ob, c):
                # fp8 weights feed the matmul stream directly (exact in bf16
                # product math; values are IEEE-e4m3 of w*scale/2) - no dequant
                w8c = w8_pool.tile([P, WCK, OBW], FP8, name=f"w8_{ob}_{c}", tag="w8")
                nc.sync.dma_start(out=w8c, in_=w_in[ob, :, c * WCK:(c + 1) * WCK, :])
                return w8c

            def emit_mms(ps_slice, mt, wchunks, kts):
                for kt in kts:
                    wb_sl = wchunks[kt // WCK][:, kt % WCK, :]
                    nc.tensor.matmul(
                        ps_slice, xt[:, mt, kt, :], wb_sl,
                        start=(kt == 0), stop=(kt == KT - 1),
                        skip_group_check=True,
                    )

            def evict(ps, bias_t, y_ap):
                # fused (psum * s2) + bias -> SBUF, then one DMA out; for
                # pairs, y_ap covers 256 rows and both banks go in one DMA
                w = ps.shape[-1]
                out_sb = out_pool.tile([P, w], F32, tag="out")
                nc.vector.scalar_tensor_tensor(
                    out_sb, ps, s2[:, :], bias_t,
                    mybir.AluOpType.mult, mybir.AluOpType.add,
                )
                if w == OBW:
                    nc.sync.dma_start(out=y_ap, in_=out_sb)
                else:
                    nc.sync.dma_start(
                        out=y_ap.rearrange("(h r) c -> r h c", h=2),
                        in_=out_sb.rearrange("p (h c) -> p h c", h=2),
                    )

            # ---- Phase A: pipelined chunk stream; ob0 matmuls lag the
            # transpose+copy stream by SKEW chunks ----
            bias0 = bias_pool.tile([P, OBW], F32, name="bias_0", tag="bias")
            nc.sync.dma_start(out=bias0, in_=b_in[:, 0:OBW])
            wch0 = []
            ps0 = {}

            # HAM warm-up: dummy transposes of the identity keep the PE busy
            # while the first x chunk is still in flight, so the clock gate
            # ramps to 8/8 before the real stream starts instead of ~25us in
            # HAM warm-up: dependency-free dummy transposes (garbage data,
            # output discarded) start right after the engine preamble and
            # keep the PE busy so the clock gate ramps to 8/8 early
            tp_w = warm_ps_pool.tile([P, 1024], BF16, name="tp_warm", tag="warm")

            def warm(n):
                # dependency-free filler transposes (WAW-only on tp_w): they
                # never wait, so they pad PE bubbles without blocking real work
                for i in range(n):
                    nc.tensor.matmul(
                        tp_w[:, (i % 8) * P:(i % 8 + 1) * P], garbage, garbage,
                        is_transpose=True, skip_group_check=True,
                    )
            warm(48)
            # first two ob0 weight chunks ahead of the loop: their dequants
            # precede the first transpose copy in the scalar queue
            wch0.append(load_wchunk(0, 0))
            wch0.append(load_wchunk(0, 1))

            def prep_chunk(g):
                mt, c = divmod(g, XC)
                if g in first_xins:
                    xin = first_xins[g]
                else:
                    xin = xin_pool.tile([P, 1024], F32, name=f"xin_{g}", tag="xin")
                    nc.sync.dma_start(
                        out=xin, in_=x_in[mt * P:(mt + 1) * P, c * 1024:(c + 1) * 1024]
                    )
                # DVE pre-cast to bf16: the PE transpose then streams 16-bit
                # (2 cols/cycle) and its PSUM tile is 1 bank instead of 2
                xbf = xbf_pool.tile([P, 1024], BF16, name=f"xbf_{g}", tag="xbf")
                nc.vector.tensor_copy(xbf, xin)
                tp = tp_ps_pool.tile([P, 1024], BF16, name=f"tp_{g}", tag="tp")
                for kk in range(8):
                    nc.tensor.matmul(
                        tp[:, kk * P:(kk + 1) * P], xbf[:, kk * P:(kk + 1) * P],
                        ident_bf, is_transpose=True, skip_group_check=True,
                    )
                # one merged copy: [128, 1024] f32 PSUM -> bf16 xt (contiguous)
                nc.scalar.copy(xt[:, mt, c * 8:(c + 1) * 8, :], tp)
                # interleave ob0 weight dequants into the early scalar stream
                if g < WCH // 2 - 1:
                    wch0.append(load_wchunk(0, 2 * g + 2))
                    wch0.append(load_wchunk(0, 2 * g + 3))

            def mm_chunk(g):
                mt, c = divmod(g, XC)
                if c == 0:
                    ps0[mt] = mm_ps_pool.tile([P, OBW], F32, name=f"ps_0_{mt}", tag="ps")
                emit_mms(ps0[mt], mt, wch0, range(c * 8, (c + 1) * 8))
                if c == XC - 1:
                    evict(ps0.pop(mt), bias0, y_out[mt * P:(mt + 1) * P, 0:OBW])

            for g in range(NCH + SKEW):
                if g < NCH:
                    prep_chunk(g)
                    if g < 4:
                        # pad the pipeline-ramp bubbles so the clock gate
                        # never dips while the first chunks stream in
                        warm(8)
                if g >= SKEW:
                    mm_chunk(g - SKEW)

            # ---- Phase B: o-blocks 1..7, m-tile pairs share one 2-bank PSUM
            # tile and one fused eviction + one pair-merged y DMA ----
            for ob in range(1, OB):
                wchunks = [load_wchunk(ob, c) for c in range(WCH)]
                bias2 = bias_pool.tile([P, 2 * OBW], F32, name=f"bias2_{ob}", tag="bias2")
                for h in range(2):
                    nc.sync.dma_start(
                        out=bias2[:, h * OBW:(h + 1) * OBW],
                        in_=b_in[:, ob * OBW:(ob + 1) * OBW],
                    )
                for mt0 in range(0, MT, 2):
                    if ob == OB - 1 and mt0 == MT - 2:
                        # last pair: two single-bank groups so the final
                        # eviction + y DMA chain at the drain is half as long
                        for mt in (mt0, mt0 + 1):
                            ps = mm_ps_pool.tile([P, OBW], F32, name=f"ps_{ob}_{mt}", tag="ps")
                            emit_mms(ps, mt, wchunks, range(KT))
                            evict(ps, bias2[:, 0:OBW],
                                  y_out[mt * P:(mt + 1) * P, ob * OBW:(ob + 1) * OBW])
                    else:
                        ps = mm_ps_pool.tile([P, 2 * OBW], F32, name=f"ps_{ob}_{mt0}", tag="ps")
                        emit_mms(ps[:, 0:OBW], mt0, wchunks, range(KT))
                        emit_mms(ps[:, OBW:2 * OBW], mt0 + 1, wchunks, range(KT))
                        evict(ps, bias2,
                              y_out[mt0 * P:(mt0 + 2) * P, ob * OBW:(ob + 1) * OBW])

    nc.finalize()
    return nc


def _get_program():
    if "nc" not in _STATE:
        _STATE["nc"] = _build_program()
    return _STATE["nc"]


def _prep_weights(weight_fp8):
    """Re-encode jax e4m3fn bytes as IEEE-e4m3 bytes of value/2 (exact for
    normals), transpose to [d, o], and block to [ob, p, kt, obw] so each
    o-block DMA reads 2KB-contiguous per-partition lines."""
    bits = np.arange(256, dtype=np.uint8)
    vals = bits.view(ml_dtypes.float8_e4m3fn).astype(np.float32) * 0.5
    lut = vals.astype(ml_dtypes.float8_e4m3).view(np.uint8)

    wb = np.asarray(weight_fp8).view(np.uint8)          # [DO, DI]
    w2t = np.ascontiguousarray(lut[wb].T)               # [DI, DO]
    w_pre = np.ascontiguousarray(
        w2t.reshape(KT, P, OB, OBW).transpose(2, 1, 0, 3)
    )                                                   # [OB, P, KT, OBW]
    return w_pre.view(ml_dtypes.float8_e4m3)


def kernel(x, weight_fp8, weight_inv_scale, bias):
    from concourse.bass_utils import run_bass_kernel_spmd

    try:
        import jax
        jax.config.update("jax_compilation_cache_dir", "/tmp/jax_neff_cache")
        jax.config.update("jax_persistent_cache_min_entry_size_bytes", 0)
        jax.config.update("jax_persistent_cache_min_compile_time_secs", 0.0)
    except Exception:
        pass

    nc = _get_program()

    x_np = np.asarray(x, dtype=np.float32).reshape(M, DI)
    w_pre = _prep_weights(weight_fp8)
    s_b = np.ascontiguousarray(
        np.broadcast_to(
            np.asarray(weight_inv_scale, dtype=np.float32).reshape(1, 1), (P, 1)
        )
    )
    b_b = np.ascontiguousarray(
        np.broadcast_to(np.asarray(bias, dtype=np.float32), (P, DO))
    )

    core_ids = list(range(NCORES))
    in_maps = [
        {"x": x_np[c * MC:(c + 1) * MC], "w": w_pre, "s": s_b, "b": b_b}
        for c in core_ids
    ]

    last_err = None
    for _attempt in range(3):
        try:
            res = run_bass_kernel_spmd(nc, in_maps, core_ids)
            break
        except Exception as e:  # device wedge (NRT_EXEC_UNIT_UNRECOVERABLE): reset + retry
            last_err = e
            try:
                import jax
                import time
                jax.clear_backends()
                time.sleep(3.0)
            except Exception:
                pass
    else:
        raise last_err

    y = np.concatenate([res.results[c]["y"] for c in core_ids], axis=0)
    return y.reshape(B, S, DO)


# revision 35
# speedup vs baseline: 1.0093x; 1.0030x over previous
"""TRN2 Bass kernel for nn_FP8LinearWrapper: y = x @ (w_fp8 * inv_scale).T + bias.

Strategy (8 NeuronCores, SPMD):
  - Data-parallel over the flattened token dim: x [4,2048,4096] -> [8192,4096],
    1024 rows per core. Weights/bias replicated to every core.
  - Per core: SINGLE-pass bf16 matmul (x cast to bf16 on device) against the
    exactly-dequantized bf16 weight, accumulating in fp32 PSUM. Error budget:
    x's bf16 rounding alone -> rel_absmax ~1.7e-3 (measured vs fp32 reference),
    well under the 2e-2 gate; the fp8 weight dequant is exact in bf16.
  - The fp8 weight bytes are jax float8_e4m3fn (max 448). TRN2's fp8e4 decode
    is IEEE e4m3 (max 240), so the host re-encodes each byte via a LUT to the
    e4m3 bits of (value/2) - exact for all normals - and the kernel folds the
    missing *2 into the output scale. Weights stay 1 byte; all arithmetic
    (dequant cast, transpose of x, matmul, scale, bias) runs on device.
  - x is transposed on device via PE-transpose (contraction dim must be on
    SBUF partitions for both matmul operands). w is passed pre-transposed /
    pre-blocked (weight layout prep, as for any serving stack).

Timing-critical structure (PE roofline: 2048 matmuls x 216 ns N=512 bf16
streaming cadence + 256 PE-transposes x ~108 ns ~ 470 us/core):
  - Phase A: a software-pipelined chunk stream: per 1024-col chunk, 8 PE
    transposes land in ONE 2-bank PSUM tile, one merged scalar copy (f32 psum
    -> bf16 xt, contiguous both sides) retires them, and the o-block-0 matmuls
    for chunk g-2 run in between (2-chunk skew) so the PE never waits on the
    scalar engine. o-block-0 weight dequants are interleaved into the first
    chunks' scalar stream to keep mt0 dense (HAM clock gate stays 8/8).
  - O-blocks 1..7: m-tile PAIRS share one 2-bank PSUM tile (2 x 32
    accumulating matmuls) with a single fused (psum * 2*inv_scale) + bias DVE
    eviction covering both banks and ONE pair-merged y DMA (256 rows).
"""

import os
import sys

for _p in (
    "/opt/trn_rl_repo",
    "/root/.axon_site",
    "/root/.axon_site/_ro/trn_rl_repo",
    "/root/.axon_site/_ro/pypackages",
):
    if os.path.isdir(_p) and _p not in sys.path:
        sys.path.append(_p)

import numpy as np
import ml_dtypes

B, S, DI, DO = 4, 2048, 4096, 4096
NCORES = 8
M = B * S            # 8192
MC = M // NCORES     # 1024 rows per core
P = 128
KT = DI // P         # 32 k-tiles
MT = MC // P         # 8 m-tiles per core
OBW = 512            # o-block width
OB = DO // OBW       # 8 o-blocks
WCK = 4              # k-tiles per weight chunk
WCH = KT // WCK      # 8 weight chunks per o-block
XC = 4               # 1024-col x chunks per m-tile
NCH = MT * XC        # 32 chunks total
SKEW = 2             # ob0 matmuls lag the transpose stream by this many chunks

_STATE = {}


def _build_program():
    import concourse.bass as bass
    import concourse.mybir as mybir
    import concourse.tile as tile
    from concourse import bacc
    from concourse.masks import make_identity

    dt = mybir.dt
    F32, BF16, FP8 = dt.float32, dt.bfloat16, dt.float8e4

    nc = bacc.Bacc(target_bir_lowering=False)

    x_in = nc.dram_tensor("x", [MC, DI], F32, kind="ExternalInput")
    w_in = nc.dram_tensor("w", [OB, P, KT, OBW], FP8, kind="ExternalInput")
    s_in = nc.dram_tensor("s", [P, 1], F32, kind="ExternalInput")
    b_in = nc.dram_tensor("b", [P, DO], F32, kind="ExternalInput")
    y_out = nc.dram_tensor("y", [MC, DO], F32, kind="ExternalOutput")

    with tile.TileContext(nc) as tc:
        with (
            tc.tile_pool(name="const", bufs=1) as const,
            tc.tile_pool(name="xt_pool", bufs=1) as xt_pool,
            tc.tile_pool(name="xin_pool", bufs=10) as xin_pool,
            tc.tile_pool(name="xbf_pool", bufs=6) as xbf_pool,
            tc.tile_pool(name="w8_pool", bufs=18) as w8_pool,
            tc.tile_pool(name="bias_pool", bufs=2) as bias_pool,
            tc.tile_pool(name="out_pool", bufs=4) as out_pool,
            tc.tile_pool(name="tp_ps_pool", bufs=2, space="PSUM") as tp_ps_pool,
            tc.tile_pool(name="warm_ps_pool", bufs=1, space="PSUM") as warm_ps_pool,
            tc.tile_pool(name="mm_ps_pool", bufs=2, space="PSUM") as mm_ps_pool,
        ):
            # x DMAs for the first chunks go first on the sync queue: they
            # head the critical chain (DMA -> cast -> transpose -> copy -> mm)
            first_xins = {}
            for g in range(2):
                xin = xin_pool.tile([P, 1024], F32, name=f"xin_{g}", tag="xin")
                nc.sync.dma_start(out=xin, in_=x_in[0:P, g * 1024:(g + 1) * 1024])
                first_xins[g] = xin

            ident = const.tile([P, P], F32)
            make_identity(nc, ident)
            ident_bf = const.tile([P, P], BF16)
            nc.vector.tensor_copy(ident_bf, ident)
            s_t = const.tile([P, 1], F32)
            nc.sync.dma_start(out=s_t, in_=s_in[:, :])
            s2 = const.tile([P, 1], F32)
            nc.scalar.mul(s2, s_t, 2.0)  # fold back the /2 from the fp8 re-encode
            garbage = const.tile([P, P], BF16)  # HAM warm-up fodder only
            nc.gpsimd.memset(garbage, 0)

            # resident transposed x: [128 d, mt, kt, 128 m] bf16
            xt = xt_pool.tile([P, MT, KT, P], BF16)

            def load_wchunk(ob, c):
                # fp8 weights feed the matmul stream directly (exact in bf16
                # product math; values are IEEE-e4m3 of w*scale/2) - no dequant
                w8c = w8_pool.tile([P, WCK, OBW], FP8, name=f"w8_{ob}_{c}", tag="w8")
                nc.sync.dma_start(out=w8c, in_=w_in[ob, :, c * WCK:(c + 1) * WCK, :])
                return w8c

            def emit_mms(ps_slice, mt, wchunks, kts):
                for kt in kts:
                    wb_sl = wchunks[kt // WCK][:, kt % WCK, :]
                    nc.tensor.matmul(
                        ps_slice, xt[:, mt, kt, :], wb_sl,
                        start=(kt == 0), stop=(kt == KT - 1),
                        skip_group_check=True,
                    )

            def evict(ps, bias_t, y_ap):
                # fused (psum * s2) + bias -> SBUF, then one DMA out; for
                # pairs, y_ap covers 256 rows and both banks go in one DMA
                w = ps.shape[-1]
                out_sb = out_pool.tile([P, w], F32, tag="out")
                nc.vector.scalar_tensor_tensor(
                    out_sb, ps, s2[:, :], bias_t,
                    mybir.AluOpType.mult, mybir.AluOpType.add,
                )
                if w == OBW:
                    nc.sync.dma_start(out=y_ap, in_=out_sb)
                else:
                    nc.sync.dma_start(
                        out=y_ap.rearrange("(h r) c -> r h c", h=2),
                        in_=out_sb.rearrange("p (h c) -> p h c", h=2),
                    )

            # ---- Phase A: pipelined chunk stream; ob0 matmuls lag the
            # transpose+copy stream by SKEW chunks ----
            bias0 = bias_pool.tile([P, OBW], F32, name="bias_0", tag="bias")
            nc.sync.dma_start(out=bias0, in_=b_in[:, 0:OBW])
            wch0 = []
            ps0 = {}

            # HAM warm-up: dummy transposes of the identity keep the PE busy
            # while the first x chunk is still in flight, so the clock gate
            # ramps to 8/8 before the real stream starts instead of ~25us in
            # HAM warm-up: dependency-free dummy transposes (garbage data,
            # output discarded) start right after the engine preamble and
            # keep the PE busy so the clock gate ramps to 8/8 early
            tp_w = warm_ps_pool.tile([P, 1024], BF16, name="tp_warm", tag="warm")

            def warm(n):
                # dependency-free filler transposes (WAW-only on tp_w): they
                # never wait, so they pad PE bubbles without blocking real work
                for i in range(n):
                    nc.tensor.matmul(
                        tp_w[:, (i % 8) * P:(i % 8 + 1) * P], garbage, garbage,
                        is_transpose=True, skip_group_check=True,
                    )
            warm(36)
            # first two ob0 weight chunks ahead of the loop: their dequants
            # precede the first transpose copy in the scalar queue
            wch0.append(load_wchunk(0, 0))
            wch0.append(load_wchunk(0, 1))

            def prep_chunk(g):
                mt, c = divmod(g, XC)
                if g in first_xins:
                    xin = first_xins[g]
                else:
                    xin = xin_pool.tile([P, 1024], F32, name=f"xin_{g}", tag="xin")
                    nc.sync.dma_start(
                        out=xin, in_=x_in[mt * P:(mt + 1) * P, c * 1024:(c + 1) * 1024]
                    )
                # DVE pre-cast to bf16: the PE transpose then streams 16-bit
                # (2 cols/cycle) and its PSUM tile is 1 bank instead of 2
                xbf = xbf_pool.tile([P, 1024], BF16, name=f"xbf_{g}", tag="xbf")
                nc.vector.tensor_copy(xbf, xin)
                tp = tp_ps_pool.tile([P, 1024], BF16, name=f"tp_{g}", tag="tp")
                for kk in range(8):
                    nc.tensor.matmul(
                        tp[:, kk * P:(kk + 1) * P], xbf[:, kk * P:(kk + 1) * P],
                        ident_bf, is_transpose=True, skip_group_check=True,
                    )
                # one merged copy: [128, 1024] f32 PSUM -> bf16 xt (contiguous)
                nc.scalar.copy(xt[:, mt, c * 8:(c + 1) * 8, :], tp)
                # interleave ob0 weight dequants into the early scalar stream
                if g < WCH // 2 - 1:
                    wch0.append(load_wchunk(0, 2 * g + 2))
                    wch0.append(load_wchunk(0, 2 * g + 3))

            def mm_chunk(g):
                mt, c = divmod(g, XC)
                if c == 0:
                    ps0[mt] = mm_ps_pool.tile([P, OBW], F32, name=f"ps_0_{mt}", tag="ps")
                emit_mms(ps0[mt], mt, wch0, range(c * 8, (c + 1) * 8))
                if c == XC - 1:
                    evict(ps0.pop(mt), bias0, y_out[mt * P:(mt + 1) * P, 0:OBW])

            for g in range(NCH + SKEW):
                if g < NCH:
                    prep_chunk(g)
                    if g < 4:
                        # pad the pipeline-ramp bubbles so the clock gate
                        # never dips while the first chunks stream in
                        warm(8)
                if g >= SKEW:
                    mm_chunk(g - SKEW)

            # ---- Phase B: o-blocks 1..7, m-tile pairs share one 2-bank PSUM
            # tile and one fused eviction + one pair-merged y DMA ----
            for ob in range(1, OB):
                wchunks = [load_wchunk(ob, c) for c in range(WCH)]
                bias2 = bias_pool.tile([P, 2 * OBW], F32, name=f"bias2_{ob}", tag="bias2")
                for h in range(2):
                    nc.sync.dma_start(
                        out=bias2[:, h * OBW:(h + 1) * OBW],
                        in_=b_in[:, ob * OBW:(ob + 1) * OBW],
                    )
                for mt0 in range(0, MT, 2):
                    if ob == OB - 1 and mt0 == MT - 2:
                        # last pair: two single-bank groups so the final
                        # eviction + y DMA chain at the drain is half as long
                        for mt in (mt0, mt0 + 1):
                            ps = mm_ps_pool.tile([P, OBW], F32, name=f"ps_{ob}_{mt}", tag="ps")
                            emit_mms(ps, mt, wchunks, range(KT))
                            evict(ps, bias2[:, 0:OBW],
                                  y_out[mt * P:(mt + 1) * P, ob * OBW:(ob + 1) * OBW])
                    else:
                        ps = mm_ps_pool.tile([P, 2 * OBW], F32, name=f"ps_{ob}_{mt0}", tag="ps")
                        emit_mms(ps[:, 0:OBW], mt0, wchunks, range(KT))
                        emit_mms(ps[:, OBW:2 * OBW], mt0 + 1, wchunks, range(KT))
                        evict(ps, bias2,
                              y_out[mt0 * P:(mt0 + 2) * P, ob * OBW:(ob + 1) * OBW])

    nc.finalize()
    return nc


def _get_program():
    if "nc" not in _STATE:
        _STATE["nc"] = _build_program()
    return _STATE["nc"]


def _prep_weights(weight_fp8):
    """Re-encode jax e4m3fn bytes as IEEE-e4m3 bytes of value/2 (exact for
    normals), transpose to [d, o], and block to [ob, p, kt, obw] so each
    o-block DMA reads 2KB-contiguous per-partition lines."""
    bits = np.arange(256, dtype=np.uint8)
    vals = bits.view(ml_dtypes.float8_e4m3fn).astype(np.float32) * 0.5
    lut = vals.astype(ml_dtypes.float8_e4m3).view(np.uint8)

    wb = np.asarray(weight_fp8).view(np.uint8)          # [DO, DI]
    w2t = np.ascontiguousarray(lut[wb].T)               # [DI, DO]
    w_pre = np.ascontiguousarray(
        w2t.reshape(KT, P, OB, OBW).transpose(2, 1, 0, 3)
    )                                                   # [OB, P, KT, OBW]
    return w_pre.view(ml_dtypes.float8_e4m3)


def kernel(x, weight_fp8, weight_inv_scale, bias):
    from concourse.bass_utils import run_bass_kernel_spmd

    try:
        import jax
        jax.config.update("jax_compilation_cache_dir", "/tmp/jax_neff_cache")
        jax.config.update("jax_persistent_cache_min_entry_size_bytes", 0)
        jax.config.update("jax_persistent_cache_min_compile_time_secs", 0.0)
    except Exception:
        pass

    nc = _get_program()

    x_np = np.asarray(x, dtype=np.float32).reshape(M, DI)
    w_pre = _prep_weights(weight_fp8)
    s_b = np.ascontiguousarray(
        np.broadcast_to(
            np.asarray(weight_inv_scale, dtype=np.float32).reshape(1, 1), (P, 1)
        )
    )
    b_b = np.ascontiguousarray(
        np.broadcast_to(np.asarray(bias, dtype=np.float32), (P, DO))
    )

    core_ids = list(range(NCORES))
    in_maps = [
        {"x": x_np[c * MC:(c + 1) * MC], "w": w_pre, "s": s_b, "b": b_b}
        for c in core_ids
    ]

    last_err = None
    for _attempt in range(3):
        try:
            res = run_bass_kernel_spmd(nc, in_maps, core_ids)
            break
        except Exception as e:  # device wedge (NRT_EXEC_UNIT_UNRECOVERABLE): reset + retry
            last_err = e
            try:
                import jax
                import time
                jax.clear_backends()
                time.sleep(3.0)
            except Exception:
                pass
    else:
        raise last_err

    y = np.concatenate([res.results[c]["y"] for c in core_ids], axis=0)
    return y.reshape(B, S, DO)
